# revision 27
# baseline (speedup 1.0000x reference)
"""Frequency-Channel-Attention kernel for Trainium2 (8 NeuronCores, SPMD), v5.

Math: dct2(X) = D @ X @ D^T with D[k,j] = cos(pi*k*(2j+1)/(2L))/L, L=64.
Per (b,c): S = max(dct2(ip[b,c])); h = relu(S@w1); z = sigmoid(h@w2);
out = ip * z[b,c].

Per-j (j = 2*b + cg, 128 channels cl) dataflow:
  - SWDGE casting loads: T16[j][half][cl, 2048] bf16 (h,w order, h = 2H+r).
  - T1 (PE): 32 chunk transposes -> psa bf16 (2 psum bufs) -> ACT evac with
    strided write -> X[(r,w), (cl, H)] (cl-major so s1 rhs is contiguous).
  - s1 (PE): lhsT = BD1[(r,w),(r',k2)] = d_rr' D[k2,w]; rhs = X natural
    512-col slices -> ps1[(r',k2), (cl64, H)] f32 (half tile, 4 banks).
  - ACT cast evac ps1 -> A bf16 [128,2048]; DVE StreamTranspose (32x32
    blocks) -> R[(r',k2h,H), (cl, k2m)].
  - s2 (PE, single-shot): lhsT = BD2p[(r',k2h,H),(k2h',k1)] =
    d_k2h D[k1, 2H+r'] -> ps2[(k2h,k1), (cl32, k2m)] f32 per quarter.
  - DVE reduce_max over k2m -> Mx[j][(k2h,k1), cl]; PE transpose of Mx
    (psum scratch from ps2 pool) + DVE reduce -> S[cl].
  - MLP on PE/ACT per batch; multiply per half: j0/j1 on gpsimd (mid),
    j2 on DVE / j3 on ACT (tail); f32 HWDGE stores on sync+scalar rings.
"""

import os
import sys

import numpy as np

for _p in ("/opt/trn_rl_repo", "/opt/pypackages"):
    if os.path.isdir(_p) and _p not in sys.path:
        sys.path.append(_p)

import concourse.bacc as bacc
import concourse.tile as tile
from concourse import mybir
from concourse.bass_utils import run_bass_kernel_spmd

F32 = mybir.dt.float32
BF16 = mybir.dt.bfloat16

B, C, H, W = 16, 256, 64, 64
N_CORES = 8
B_LOC = B // N_CORES
NJ = 4  # j = 2*b + cg

_NC_CACHE = {}


def _dct_matrix():
    k = np.arange(W, dtype=np.float64)[:, None]
    j = np.arange(W, dtype=np.float64)[None, :]
    D = np.cos(np.pi * k * (2.0 * j + 1.0) / (2.0 * W)) / W
    return D.astype(np.float32)


def _constants():
    D = _dct_matrix()
    BD1 = np.zeros((128, 128), dtype=np.float32)  # [(r,w), (r',k2)]
    for r in range(2):
        BD1[64 * r : 64 * r + 64, 64 * r : 64 * r + 64] = D.T
    BD2p = np.zeros((128, 128), dtype=np.float32)  # [(r',k2h,hb), (k2h',k1)]
    for rp in range(2):
        for k2h in range(2):
            base = 32 * (2 * rp + k2h)
            BD2p[base : base + 32, 64 * k2h : 64 * k2h + 64] = D.T[rp::2, :]
    identf = np.eye(128, dtype=np.float32)
    return BD1, BD2p, identf


def _build_nc():
    nc = bacc.Bacc(None, target_bir_lowering=False)
    ip_d = nc.dram_tensor("ip", [B_LOC, C, H, W], F32, kind="ExternalInput")
    w1a_d = nc.dram_tensor("w1a", [128, 16], F32, kind="ExternalInput")
    w1b_d = nc.dram_tensor("w1b", [128, 16], F32, kind="ExternalInput")
    w2_d = nc.dram_tensor("w2", [16, C], F32, kind="ExternalInput")
    bd1_d = nc.dram_tensor("bd1", [128, 128], BF16, kind="ExternalInput")
    bd2_d = nc.dram_tensor("bd2p", [128, 128], BF16, kind="ExternalInput")
    idf_d = nc.dram_tensor("identf", [128, 128], F32, kind="ExternalInput")
    idb_d = nc.dram_tensor("identb", [128, 128], BF16, kind="ExternalInput")
    out_d = nc.dram_tensor("out", [B_LOC, C, H, W], F32, kind="ExternalOutput")

    from contextlib import ExitStack

    with tile.TileContext(nc) as tc, ExitStack() as ctx:
        const = ctx.enter_context(tc.tile_pool(name="const", bufs=1))
        big = ctx.enter_context(tc.tile_pool(name="big", bufs=1))
        xp = ctx.enter_context(tc.tile_pool(name="xp", bufs=2))
        rfpool = ctx.enter_context(tc.tile_pool(name="rf", bufs=3))
        rpool = ctx.enter_context(tc.tile_pool(name="rp", bufs=6))
        mxp = ctx.enter_context(tc.tile_pool(name="mxp", bufs=2))
        o32p = ctx.enter_context(tc.tile_pool(name="o32", bufs=3))
        misc = ctx.enter_context(tc.tile_pool(name="misc", bufs=1))
        psap = ctx.enter_context(tc.tile_pool(name="psa", bufs=2, space="PSUM"))
        ps1p = ctx.enter_context(tc.tile_pool(name="ps1", bufs=1, space="PSUM"))
        ps2p = ctx.enter_context(tc.tile_pool(name="ps2", bufs=2, space="PSUM"))

        def load_const(name_d, shape, tag):
            t = const.tile(shape, F32, tag=tag)
            nc.sync.dma_start(out=t, in_=name_d[:, :])
            return t

        def load_const_b(name_d, shape, tag):
            t = const.tile(shape, BF16, tag=tag)
            nc.sync.dma_start(out=t, in_=name_d[:, :])
            return t

        IDTb = load_const_b(idb_d, [128, 128], "idb")
        BD1 = load_const_b(bd1_d, [128, 128], "bd1b")
        BD2 = load_const_b(bd2_d, [128, 128], "bd2b")
        IDTf = load_const(idf_d, [128, 128], "idf")
        W1A = load_const(w1a_d, [128, 16], "w1a")
        W1B = load_const(w1b_d, [128, 16], "w1b")
        W2t = load_const(w2_d, [16, 256], "w2t")

        # ---- bf16 input via gpsimd SWDGE casting loads (8KB descs) ----
        ip_f = ip_d.rearrange("b (cg cl) h w -> cl b cg (h w)", cg=2)
        out_v = out_d.rearrange("b (cg cl) h w -> cl b cg (h w)", cg=2)
        T16 = []
        for j in range(NJ):
            b, cg = j // 2, j % 2
            halves = []
            for half in range(2):
                t = big.tile([128, 2048], BF16, tag=f"t16_{j}_{half}")
                for qt in range(2):
                    nc.gpsimd.dma_start(
                        out=t[:, 1024 * qt : 1024 * (qt + 1)],
                        in_=ip_f[
                            :, b, cg,
                            2048 * half + 1024 * qt : 2048 * half + 1024 * (qt + 1),
                        ],
                    )
                halves.append(t)
            T16.append(halves)


        Scols = misc.tile([128, NJ], F32)
        hT = misc.tile([16, 2], F32)
        Zpp = misc.tile([128, NJ], F32)

        Xs = [None] * NJ
        Xvs = [None] * NJ
        Mxs = [None] * NJ
        Rs = {}

        def t1_start(j):
            X = xp.tile([128, 4096], BF16, tag="x", name=f"x{j}")
            Xs[j] = X
            Xvs[j] = X.rearrange("p (hc c) -> p hc c", hc=32)
            Mxs[j] = mxp.tile([128, 128], F32, tag="mx", name=f"mx{j}")

        def t1_batch(j, a):
            # X[(r,w), (Hc, cl)]: natural chunk-transpose order, contiguous
            X = Xs[j]
            psa = psap.tile([128, 1024], BF16, tag="psa")
            for t in range(8):
                Hc = 8 * a + t
                nc.tensor.transpose(
                    psa[:, 128 * t : 128 * t + 128],
                    T16[j][Hc // 16][:, 128 * (Hc % 16) : 128 * (Hc % 16) + 128],
                    IDTb,
                )
            nc.scalar.copy(out=X[:, 1024 * a : 1024 * (a + 1)], in_=psa)

        def t1(j):
            t1_start(j)
            for a in range(4):
                t1_batch(j, a)

        def s1q(j, q):
            """s1 matmuls for one 32-channel quarter + StreamT + cast.

            rhs iterates (Hc outer, cl-16 inner): 32B-contiguous runs.
            ps1 free = (m2, Hc32, cl16).  T2 = StreamT with a strided input
            view reordering to (m, cl16, Hc) so view-blocks are (cl',
            within=Hc); output Rf natural (cl', k2m).  ACT casts to bf16.
            """
            ps1 = ps1p.tile([128, 1024], F32, tag="ps1")
            for hh in range(2):
                for m in range(2):
                    cl0 = 32 * q + 16 * m
                    nc.tensor.matmul(
                        ps1[:, 512 * m + 256 * hh : 512 * m + 256 * hh + 256],
                        lhsT=BD1,
                        rhs=Xvs[j][:, 16 * hh : 16 * hh + 16, cl0 : cl0 + 16],
                        start=True,
                        stop=True,
                    )
            Rf = rfpool.tile([128, 1024], F32, tag="rf")
            nc.vector.transpose(
                out=Rf.rearrange("p (m c hh hc) -> p m c hh hc", m=2, c=16, hh=2),
                in_=ps1.rearrange("p (m hh hc c) -> p m c hh hc", m=2, hh=2, hc=16),
            )
            R = rpool.tile([128, 1024], BF16, tag="r")
            nc.scalar.copy(out=R, in_=Rf)
            Rs[(j, q)] = R

        def s2q(j, q):
            R = Rs.pop((j, q))
            ps2 = ps2p.tile([128, 1024], F32, tag="ps2")
            for m in range(2):
                nc.tensor.matmul(
                    ps2[:, 512 * m : 512 * m + 512],
                    lhsT=BD2,
                    rhs=R[:, 512 * m : 512 * m + 512],
                    start=True,
                    stop=True,
                )
            nc.vector.reduce_max(
                out=Mxs[j][:, 32 * q : 32 * q + 32],
                in_=ps2.rearrange("p (c k) -> p c k", k=32),
                axis=mybir.AxisListType.X,
            )

        def fin(j):
            scrt = ps2p.tile([128, 1024], F32, tag="ps2", name=f"scr{j}")
            mxt = scrt[:, 0:128]
            nc.tensor.transpose(mxt, Mxs[j], IDTf)
            nc.vector.reduce_max(
                out=Scols[:, j : j + 1], in_=mxt, axis=mybir.AxisListType.X
            )

        def phase_b(b):
            scrt = ps2p.tile([128, 1024], F32, tag="ps2", name=f"scrb{b}")
            ph = scrt[0:16, 128:129]
            nc.tensor.matmul(
                ph, lhsT=W1A, rhs=Scols[:, 2 * b : 2 * b + 1],
                start=True, stop=False,
            )
            nc.tensor.matmul(
                ph, lhsT=W1B, rhs=Scols[:, 2 * b + 1 : 2 * b + 2],
                start=False, stop=True,
            )
            nc.scalar.activation(
                out=hT[:, b : b + 1], in_=ph,
                func=mybir.ActivationFunctionType.Relu,
            )
            pz = scrt[:, 132:134]
            for cg in range(2):
                nc.tensor.matmul(
                    pz[:, cg : cg + 1],
                    lhsT=W2t[:, 128 * cg : 128 * cg + 128],
                    rhs=hT[:, b : b + 1],
                    start=True,
                    stop=True,
                )
            nc.scalar.activation(
                out=Zpp[:, 2 * b : 2 * b + 2], in_=pz,
                func=mybir.ActivationFunctionType.Sigmoid,
            )

        def mult_store_half(j, half, eng, ring):
            b, cg = j // 2, j % 2
            o = o32p.tile([128, 2048], F32, tag="o32")
            src = T16[j][half]
            if eng == "gp":
                nc.gpsimd.tensor_tensor(
                    out=o,
                    in0=src,
                    in1=Zpp[:, j : j + 1].broadcast_to([128, 2048]),
                    op=mybir.AluOpType.mult,
                )
            elif eng == "dve":
                nc.vector.tensor_scalar_mul(o, src, Zpp[:, j : j + 1])
            else:
                nc.scalar.mul(out=o, in_=src, mul=Zpp[:, j : j + 1])
            ring.dma_start(
                out=out_v[:, b, cg, 2048 * half : 2048 * (half + 1)], in_=o
            )

        def mult_store_quarter(j, qt, eng, ring):
            # tail-optimized: 1024-col granules so stores start sooner
            b, cg = j // 2, j % 2
            half, qh = qt // 2, qt % 2
            o = o32p.tile([128, 1024], F32, tag="o32q")
            src = T16[j][half][:, 1024 * qh : 1024 * (qh + 1)]
            if eng == "dve":
                nc.vector.tensor_scalar_mul(o, src, Zpp[:, j : j + 1])
            else:
                nc.scalar.mul(out=o, in_=src, mul=Zpp[:, j : j + 1])
            ring.dma_start(
                out=out_v[:, b, cg, 2048 * half + 1024 * qh : 2048 * half + 1024 * (qh + 1)],
                in_=o,
            )

        # ---- software-pipelined emission: 2-quarter lag between s1 and
        # s2 of the same j; fin/MLP fire as early as possible ----
        t1(0)
        s1q(0, 0)
        s1q(0, 1)
        s1q(0, 2)
        s2q(0, 0)
        s1q(0, 3)
        s2q(0, 1)
        t1(1)
        s1q(1, 0)
        s2q(0, 2)
        s1q(1, 1)
        s2q(0, 3)
        fin(0)
        s1q(1, 2)
        s2q(1, 0)
        s1q(1, 3)
        s2q(1, 1)
        t1(2)
        s1q(2, 0)
        s2q(1, 2)
        s1q(2, 1)
        s2q(1, 3)
        fin(1)
        s1q(2, 2)
        s2q(2, 0)
        s1q(2, 3)
        s2q(2, 1)
        phase_b(0)
        mult_store_half(0, 0, "gp", nc.scalar)
        mult_store_half(0, 1, "gp", nc.scalar)
        t1(3)
        s1q(3, 0)
        s2q(2, 2)
        s1q(3, 1)
        s2q(2, 3)
        fin(2)
        mult_store_half(1, 0, "gp", nc.sync)
        mult_store_half(1, 1, "gp", nc.sync)
        s1q(3, 2)
        s2q(3, 0)
        s1q(3, 3)
        s2q(3, 1)
        s2q(3, 2)
        s2q(3, 3)
        fin(3)
        phase_b(1)
        mult_store_quarter(2, 0, "dve", nc.sync)
        mult_store_quarter(3, 0, "act", nc.scalar)
        mult_store_quarter(2, 1, "dve", nc.sync)
        mult_store_quarter(3, 1, "act", nc.scalar)
        mult_store_half(2, 1, "dve", nc.sync)
        mult_store_half(3, 1, "act", nc.scalar)

    nc.finalize()
    return nc


def get_nc():
    if "nc" not in _NC_CACHE:
        _NC_CACHE["nc"] = _build_nc()
    return _NC_CACHE["nc"]


def make_in_map(ip_shard, w1, w2):
    import ml_dtypes

    BD1, BD2p, identf = _constants()
    return {
        "ip": np.ascontiguousarray(ip_shard, dtype=np.float32),
        "w1a": np.ascontiguousarray(w1[0:128], dtype=np.float32),
        "w1b": np.ascontiguousarray(w1[128:256], dtype=np.float32),
        "w2": np.ascontiguousarray(w2, dtype=np.float32),
        "bd1": BD1.astype(ml_dtypes.bfloat16),
        "bd2p": BD2p.astype(ml_dtypes.bfloat16),
        "identf": identf,
        "identb": identf.astype(ml_dtypes.bfloat16),
    }


def kernel(ip, w1, w2):
    assert ip.shape == (B, C, H, W), ip.shape
    nc = get_nc()
    ip = np.ascontiguousarray(ip, dtype=np.float32)
    w1 = np.asarray(w1, dtype=np.float32)
    w2 = np.asarray(w2, dtype=np.float32)
    in_maps = [
        make_in_map(ip[B_LOC * k : B_LOC * (k + 1)], w1, w2)
        for k in range(N_CORES)
    ]
    res = run_bass_kernel_spmd(nc, in_maps, list(range(N_CORES)), **RUN_KWARGS)
    LAST_RESULT.clear()
    LAST_RESULT["exec_time_ns"] = res.exec_time_ns
    LAST_RESULT["profile_json"] = res.profile_json
    return np.concatenate([m["out"] for m in res.results], axis=0)


RUN_KWARGS = {}
LAST_RESULT = {}
